# revision 27
# baseline (speedup 1.0000x reference)
# Multi-head attention (N=2, S=2048, E=2048, H=16, Dk=128) on 8 NeuronCores.
#
# Sharding: 2 batches x 16 heads = 32 (n,h) pairs -> core c owns batch c//4,
# heads (c%4)*4 .. +4. The reference reshapes (N,H,S,Dk)->(N,S,H*Dk) without
# a head transpose, so rows [h*128,(h+1)*128) of the pre-projection matrix X
# (and hence of the final output) depend on head h only: each core computes
# 512 disjoint output rows and the host concatenates. No collectives.
#
# v18 design - 404us on HW (v3 fp32r baseline: 542us). Where time goes:
# startup ~17us (DMA lead-in), projections ~170us (PE-bound), attention
# ~135us (Act/exp-bound, PE 93% busy under it), O-proj ~62us (PE-bound),
# drain ~8us. PE active ~368us vs 348us pure-stream floor.
#
#  - every matmul operand is fp16: same 1 cycle/row PE rate as fp32r and
#    bf16 (cost model: >=256 free dim), half the DMA/SBUF of fp32, 8x
#    finer mantissa than bf16. End-to-end mean rel err 2.1e-3 (bf16 gave
#    1.7e-2, right at the gate). PSUM accumulation is fp32 throughout.
#  - host pretiling: x inputs stream as [128p, k, 512s] chunk DMAs with
#    16KB contiguous per partition; weights are one [128, k, cols] DMA
#    each; ~50 DMA instructions total vs ~350 in v3 (each costs
#    0.6-0.8us of sequencer issue time).
#  - DMA completion-order control: queue packets stripe across all 16
#    DMA engines, so any concurrent transfer delays an earlier one's
#    completion semaphore, and the tile scheduler hoists every DMA whose
#    deps allow. Each x chunk is therefore pinned behind the previous
#    chunk's data via marker copies (one per DMA region, reading the
#    matching region of the predecessor), wk/wv are pinned behind xq0,
#    and the Wo prefetch is pinned behind the last v-group so it lands
#    in the attention phase where HBM is otherwise idle. Chunk 0 is
#    split across three queues for the fastest possible start.
#  - no DRAM spill: attention output oc and all of Wo (8MB fp16) stay in
#    SBUF; the O-projection runs with zero weight DMA.
#  - softmax denominator per (h,c) pair: two pairwise-add levels over
#    the 16 exp tiles (DVE 4+4 ops, GpSimd 4 ops - GpSimd cannot read
#    PSUM and costs ~1.3us per [128,512] op), then a 4-matmul PE chain
#    with a full ones[128,128] stationary, which yields the column sum
#    ALREADY BROADCAST to every partition (no [1,N] tile, no copy, no
#    separate broadcast matmul).
#  - bv is folded into the division epilogue: softmax rows sum to 1, so
#    attn@(v+bv) = attn@v + bv, and in the oc[d,h,s] layout bv is a
#    per-partition scalar -> one DVE tensor_scalar_add, zero PE cost.
#  - exp as [128,1024] ops over 2-bank PSUM score pairs; scores are
#    triple-buffered (6 banks) so PE is never gated by Act's drain rate;
#    outT/dbc accumulators are single-buffered (lifetimes don't overlap).
#    Act is the attention-phase bottleneck at ~8.3us/pair vs PE 7.7us.
#  - the first head-chunk's scores+exp interleave into the v-projection
#    to warm Act's pipeline (and its spline table loads at t=0 via a
#    warmup exp).
#  - O-proj loops h -> k -> nn: stationary (strided oc k-slice) loaded
#    once per (h,k), reused for 4 moving Wo tiles; bias via ones-row
#    matmul; outputs stream per 512-wide chunk.
#  - pe pstate is real: matmuls run ~634ns (1.2GHz) until ~3us of
#    continuous execution, so long stalls cost extra through re-ramp.
import numpy as np

F16 = np.float16

D_MODEL = 2048
NHEAD = 16
DK = 128
N_BATCH = 2
SEQ = 2048
N_CORES = 8
HEADS_PER_CORE = 4


class Cfg:
    def __init__(self, S=SEQ, E=D_MODEL, NH=HEADS_PER_CORE, CH=512):
        assert S % 128 == 0 and E % 128 == 0
        self.S = S          # sequence length
        self.E = E          # model dim (contraction for projections)
        self.NH = NH        # heads per core
        self.CH = CH        # s-chunk width for attention phase
        self.NK = E // 128  # contraction tiles for projections / O-proj
        self.NT = S // 128  # t tiles (attention contraction)
        self.HDc = NH * DK  # head dims per core
        self.NCH = S // CH  # number of attention s-chunks
        self.PCH = 512      # projection / O-proj free-dim chunk
        self.NPC = S // self.PCH   # projection s-chunks
        self.NOC = E // self.PCH   # O-proj output chunks
        assert S % CH == 0 and CH >= 256


def build_program(cfg: Cfg):
    import concourse.tile as tile
    from concourse import bacc, mybir
    from contextlib import ExitStack

    fp32 = mybir.dt.float32
    fp16 = mybir.dt.float16
    AF = mybir.ActivationFunctionType

    S, E, NH, CH = cfg.S, cfg.E, cfg.NH, cfg.CH
    NK, NT, HDc = cfg.NK, cfg.NT, cfg.HDc
    PCH, NPC, NOC, NCH = cfg.PCH, cfg.NPC, cfg.NOC, cfg.NCH
    inv_sqrt_dk = 1.0 / float(np.sqrt(DK))
    KH = NK // 2   # k-tiles per half DMA

    nc = bacc.Bacc("TRN2", target_bir_lowering=False, debug=False,
                   num_devices=N_CORES)

    # DRAM I/O (per-core values supplied via in_maps).
    # x inputs pretiled [s_chunk, partition, k, col]; weights [p, k, cols].
    xq = nc.dram_tensor("xq", [NPC, 128, NK, PCH], fp16,
                        kind="ExternalInput").ap()
    xk = nc.dram_tensor("xk", [NPC, 128, NK, PCH], fp16,
                        kind="ExternalInput").ap()
    xv = nc.dram_tensor("xv", [NPC, 128, NK, PCH], fp16,
                        kind="ExternalInput").ap()
    wq = nc.dram_tensor("wq", [128, NK, HDc], fp16, kind="ExternalInput").ap()
    wk = nc.dram_tensor("wk", [128, NK, HDc], fp16, kind="ExternalInput").ap()
    wv = nc.dram_tensor("wv", [128, NK, HDc], fp16, kind="ExternalInput").ap()
    wo = nc.dram_tensor("wo", [128, NK, E], fp16, kind="ExternalInput").ap()
    bq = nc.dram_tensor("bq", [128, NH], fp32, kind="ExternalInput").ap()
    bk = nc.dram_tensor("bk", [128, NH], fp32, kind="ExternalInput").ap()
    bv = nc.dram_tensor("bv", [128, NH], fp32, kind="ExternalInput").ap()
    onf = nc.dram_tensor("onf", [128, 128], fp16, kind="ExternalInput").ap()
    out = nc.dram_tensor("out", [NH * 128, E], fp32,
                         kind="ExternalOutput").ap()

    with tile.TileContext(nc) as tc, ExitStack() as top:
        persist = top.enter_context(tc.tile_pool(name="persist", bufs=1))
        qc = persist.tile([128, NH, S], fp16, name="qc")   # qT: [d, h, s]
        kc = persist.tile([128, NH, S], fp16, name="kc")   # kT: [d, h, s]
        vc = persist.tile([128, NT, HDc], fp16, name="vc")  # [t_p, t_t, hd]
        # Wo cache in two halves: the second half's SBUF is only
        # allocated from phase B on (after the xin pool closes)
        wop = top.enter_context(tc.tile_pool(name="wop", bufs=1))
        wo_a = wop.tile([128, NK // 2, E], fp16, name="wo_a")

        consts = top.enter_context(tc.tile_pool(name="consts", bufs=1))
        ones_sb = consts.tile([128, 128], fp16, name="ones")
        bq_sb = consts.tile([128, NH], fp32, name="bq")
        bk_sb = consts.tile([128, NH], fp32, name="bk")
        bv_sb = consts.tile([128, NH], fp32, name="bv")
        warm = consts.tile([128, 1], fp32, name="warm")

        # SBUF/PSUM pools release LIFO per side; phase-spanning pools go on
        # the right side so phase-local left pools can close under them.
        es_a = ExitStack()   # xin (left)
        es_qk = ExitStack()  # wq/wk sbuf + qk psum (left)
        es_b = ExitStack()   # expp + scores psum (right; A2 through B)
        es_v = ExitStack()   # wv sbuf + v psum (left)
        es_oc = ExitStack()  # oc + attn smalls (left; B + C)
        es_ps = ExitStack()  # ot/dn psum (left; B only)
        es_c = ExitStack()   # osb + o-proj psum (right)

        # ============== Phase A1: q/k projections ==============
        xin = es_a.enter_context(tc.tile_pool(name="xin", bufs=3))
        wqk = es_qk.enter_context(tc.tile_pool(name="wqk", bufs=1))
        pa = es_qk.enter_context(tc.tile_pool(name="pa", bufs=2,
                                              space="PSUM"))
        wq_sb = wqk.tile([128, NK, HDc], fp16, name="wq_sb")
        wk_sb = wqk.tile([128, NK, HDc], fp16, name="wk_sb")
        scrap = pa.tile([128, PCH], fp32, tag="pa0", name="scrap")
        for _ in range(30):
            nc.tensor.matmul(scrap[:, :128], ones_sb[:], ones_sb[:],
                             start=True, stop=True)
        # only wq + xq0 + consts move at t=0: everything else is pinned
        # behind xq0's arrival (marker copies) so the first s-chunk's data
        # isn't delayed by unrelated packets striped onto the same engines
        for hf in range(2):
            ks = slice(hf * KH, (hf + 1) * KH)
            nc.gpsimd.dma_start(wq_sb[:, ks, :], wq[:, ks, :])
        for t, d in ((ones_sb, onf), (bq_sb, bq), (bk_sb, bk),
                     (bv_sb, bv)):
            nc.gpsimd.dma_start(t[:], d)
        # trigger the Exp table load while the weight/x DMAs stream
        nc.scalar.activation(warm[:], bq_sb[:, :1], AF.Exp)

        xeng = [nc.sync, nc.scalar, nc.gpsimd]
        XSPLIT = [0, 6, 11, NK]   # thirds across three queues
        pin_hist = []

        def load_x(x_d, s):
            # chunk 0 is split over three queues (fastest possible start);
            # later chunks are ONE full DMA each (16KB contiguous per
            # partition -> 16KB packets, ~2x the per-packet bandwidth of
            # half-chunk 8KB packets), alternating queues, each pinned
            # behind the previous chunk's data - queue packets stripe over
            # shared engines, so unordered chunks delay each other's
            # completion semaphores.
            t = xin.tile([128, NK, PCH], fp16, tag="xin")
            if not pin_hist:
                for i in range(3):
                    ks = slice(XSPLIT[i], XSPLIT[i + 1])
                    xeng[i].dma_start(t[:, ks, :], x_d[s][:, ks, :])
            else:
                nc.vector.tensor_copy(t[:1, 0, :2], pin_hist[-1][:1, 0, :2])
                xeng[len(pin_hist) % 2].dma_start(t[:], x_d[s])
            pin_hist.append(t)
            return t

        def proj_qk(x_d, w_sb, bias_sb, dst):
            for s in range(NPC):
                first = not pin_hist
                xt = load_x(x_d, s)
                if first:
                    for i in range(3):
                        nc.vector.tensor_copy(
                            wk_sb[:1, XSPLIT[i], :2], xt[:1, XSPLIT[i], :2])
                    for hf in range(2):
                        ks = slice(hf * KH, (hf + 1) * KH)
                        nc.gpsimd.dma_start(wk_sb[:, ks, :], wk[:, ks, :])

                ps = [pa.tile([128, PCH], fp32, tag=f"pa{m}", name=f"pa{m}")
                      for m in range(NH)]
                for m in range(NH):
                    for k in range(NK):
                        nc.tensor.matmul(
                            ps[m][:], w_sb[:, k, m * 128:(m + 1) * 128],
                            xt[:, k, :], start=(k == 0), stop=(k == NK - 1))
                for m in range(NH):
                    # GpSimd cannot read PSUM; evict on DVE + Act
                    if m % 2 == 0:
                        nc.vector.tensor_scalar_add(
                            dst[:, m, s * PCH:(s + 1) * PCH], ps[m][:],
                            bias_sb[:, m:m + 1])
                    else:
                        nc.scalar.activation(
                            dst[:, m, s * PCH:(s + 1) * PCH], ps[m][:],
                            AF.Identity, bias=bias_sb[:, m:m + 1])

        proj_qk(xq, wq_sb, bq_sb, qc)
        proj_qk(xk, wk_sb, bk_sb, kc)
        es_qk.close()

        # pools that span A2 + all of phase B
        expp = es_b.enter_context(tc.tile_pool(name="expp", bufs=2,
                                               side="right"))
        stp = [None]  # scores psum pool: tmp 4-bank pool in A2, 6-bank in B

        def scores_exp(h, c):
            cs = slice(c * CH, (c + 1) * CH)
            expT = expp.tile([128, NT, CH], fp16, tag="expT",
                             name=f"expT_{h}_{c}")
            for tp in range(NT // 2):
                ps = stp[0].tile([128, 2 * CH], fp32, tag="st", name="st")
                for half in range(2):
                    tt = tp * 2 + half
                    nc.tensor.matmul(
                        ps[:, half * CH:(half + 1) * CH],
                        kc[:, h, tt * 128:(tt + 1) * 128],
                        qc[:, h, cs], start=True, stop=True)
                nc.scalar.activation(
                    expT[:, tp * 2:tp * 2 + 2, :].rearrange(
                        "p a b -> p (a b)"),
                    ps[:], AF.Exp, scale=inv_sqrt_dk)
            return expT

        # ===== Phase A2: v projection, first scores interleaved =====
        wvp = es_v.enter_context(tc.tile_pool(name="wvp", bufs=1))
        vps = es_v.enter_context(tc.tile_pool(name="vps", bufs=1,
                                              space="PSUM"))
        stp[0] = es_v.enter_context(tc.tile_pool(name="st0", bufs=2,
                                                 space="PSUM"))
        wv_sb = wvp.tile([128, NK, HDc], fp16, name="wv_sb")
        for hf in range(2):
            nc.vector.tensor_copy(wv_sb[:1, hf * KH, :2],
                                  pin_hist[-1][:1, 0, :2])
            ks = slice(hf * KH, (hf + 1) * KH)
            nc.gpsimd.dma_start(wv_sb[:, ks, :], wv[:, ks, :])

        def proj_v_group(g):
            xt = load_x(xv, g)
            ps = [vps.tile([128, HDc], fp32, tag=f"pv{j}", name=f"pv{j}")
                  for j in range(4)]
            for k in range(NK):
                for j in range(4):
                    nc.tensor.matmul(
                        ps[j][:], xt[:, k, j * 128:(j + 1) * 128],
                        wv_sb[:, k, :], start=(k == 0), stop=(k == NK - 1))
            for j in range(4):
                if j % 2 == 0:
                    nc.vector.tensor_copy(vc[:, g * 4 + j, :], ps[j][:])
                else:
                    nc.scalar.activation(vc[:, g * 4 + j, :], ps[j][:],
                                         AF.Identity)

        proj_v_group(0)
        proj_v_group(1)
        pend = [(0, 0, scores_exp(0, 0))]   # warm Act during v-proj tail
        proj_v_group(2)
        proj_v_group(3)
        es_v.close()
        es_a.close()
        stp[0] = es_b.enter_context(tc.tile_pool(name="st2", bufs=3,
                                                 space="PSUM", side="right"))

        # ============== Phase B: attention ==============
        ocp = es_oc.enter_context(tc.tile_pool(name="ocp", bufs=1))
        oc = ocp.tile([128, NH, S], fp16, name="oc")  # attn out [d, h, s]
        wo_b = ocp.tile([128, NK // 2, E], fp16, name="wo_b")
        bsc = es_oc.enter_context(tc.tile_pool(name="bsc", bufs=2))
        otp = es_ps.enter_context(tc.tile_pool(name="otp", bufs=1,
                                               space="PSUM"))
        dnp = es_ps.enter_context(tc.tile_pool(name="dnp", bufs=1,
                                               space="PSUM"))

        # prefetch all of Wo (8MB fp16) during attention. The marker copy
        # (depends on vc's last v-group) pins the DMAs to the start of
        # phase B - without it the scheduler hoists them into phase A's
        # DMA window where they fight the x loads for HBM bandwidth.
        woq = [nc.sync, nc.gpsimd, nc.sync, nc.gpsimd]
        for i in range(4):
            dst = wo_a if i < 2 else wo_b
            j = i % 2
            nc.vector.tensor_copy(dst[:1, j * 4, :2], vc[:1, NT - 1, :2])
            woq[i].dma_start(dst[:, j * 4:(j + 1) * 4, :],
                             wo[:, i * 4:(i + 1) * 4, :])

        def attn_tail(h, c, expT):
            cs = slice(c * CH, (c + 1) * CH)
            op = otp.tile([128, CH], fp32, tag="ot", name="ot")
            for tt in range(NT):
                nc.tensor.matmul(
                    op[:], vc[:, tt, h * 128:(h + 1) * 128],
                    expT[:, tt, :], start=(tt == 0), stop=(tt == NT - 1))
            # denominator: 2-level pairwise reduce (8 then 4 tiles), then
            # a 4-matmul full-ones chain = column sum broadcast to all
            # partitions in PSUM
            acc = bsc.tile([128, 8, CH], fp16, tag="acc", name="acc")
            for i in range(4):
                nc.vector.tensor_add(acc[:, i, :], expT[:, 2 * i, :],
                                     expT[:, 2 * i + 1, :])
            for i in range(4, 8):
                nc.gpsimd.tensor_add(acc[:, i, :], expT[:, 2 * i, :],
                                     expT[:, 2 * i + 1, :])
            for j in range(4):
                nc.vector.tensor_add(acc[:, j, :], acc[:, 2 * j, :],
                                     acc[:, 2 * j + 1, :])
            dbc = dnp.tile([128, CH], fp32, tag="dbc", name="dbc")
            for j in range(4):
                nc.tensor.matmul(dbc[:], ones_sb[:], acc[:, j, :],
                                 start=(j == 0), stop=(j == 3))
            rsc = bsc.tile([128, CH], fp32, tag="rsc", name="rsc")
            nc.vector.reciprocal_approx_fast(rsc[:], dbc[:])
            nc.vector.tensor_mul(oc[:, h, cs], op[:], rsc[:])
            nc.vector.tensor_scalar_add(oc[:, h, cs], oc[:, h, cs],
                                        bv_sb[:, h:h + 1])

        pairs = [(h, c) for h in range(NH) for c in range(NCH)]
        for h, c in pairs[1:]:
            expT = scores_exp(h, c)
            attn_tail(*pend.pop())
            pend.append((h, c, expT))
        attn_tail(*pend.pop())
        es_ps.close()
        es_b.close()

        # ============== Phase C: output projection ==============
        osb = es_c.enter_context(tc.tile_pool(name="osb", bufs=2,
                                              side="right"))
        pcp = es_c.enter_context(tc.tile_pool(name="pcp", bufs=2,
                                              space="PSUM", side="right"))
        for h in range(NH):
            ops = [pcp.tile([128, PCH], fp32, tag=f"pc{nn}", name=f"pc{nn}")
                   for nn in range(NOC)]
            # lhsT = X_h^T k-tile: strided view of oc (s = j*NK + k)
            lhs_h = oc[:, h, :].rearrange("p (j i) -> p i j", i=NK)
            for k in range(NK):
                wt = wo_a if k < NK // 2 else wo_b
                kk = k % (NK // 2)
                for nn in range(NOC):
                    nc.tensor.matmul(
                        ops[nn][:], lhs_h[:, k, :],
                        wt[:, kk, nn * PCH:(nn + 1) * PCH],
                        start=(k == 0), stop=(k == NK - 1))
            ot = osb.tile([128, E], fp32, tag="osb", name="osb")
            for nn in range(NOC):
                ns = slice(nn * PCH, (nn + 1) * PCH)
                if nn % 2 == 0:
                    nc.vector.tensor_copy(ot[:, ns], ops[nn][:])
                else:
                    nc.scalar.activation(ot[:, ns], ops[nn][:], AF.Identity)
                nc.sync.dma_start(out[h * 128:(h + 1) * 128, ns],
                                  ot[:, ns])
        es_c.close()
        es_oc.close()

    nc.compile()
    return nc


def _tile_x(x):
    # (S, E) -> [s_chunk, partition, k, col] fp16, 16KB contiguous per
    # partition per chunk
    S, E = x.shape
    return np.ascontiguousarray(
        x.reshape(S // 512, 512, E // 128, 128).transpose(0, 3, 2, 1)
    ).astype(F16)


def _tile_w(w_slice):
    # (HDc, E) -> [partition, k, col] fp16
    HDc, E = w_slice.shape
    return np.ascontiguousarray(
        w_slice.T.reshape(E // 128, 128, HDc).transpose(1, 0, 2)
    ).astype(F16)


def shard_inputs(cfg: Cfg, query, key, value, Wq, bq, Wk, bk, Wv, bv, Wo, bo):
    """Build per-core in_maps from full inputs."""
    f = np.float32
    query, key, value = (np.asarray(a, f) for a in (query, key, value))
    Wq, Wk, Wv, Wo = (np.asarray(a, f) for a in (Wq, Wk, Wv, Wo))
    bq, bk, bv, bo = (np.asarray(a, f) for a in (bq, bk, bv, bo))
    NH, HDc, NK, E = cfg.NH, cfg.HDc, cfg.NK, cfg.E
    xq_t = [_tile_x(query[n]) for n in range(N_BATCH)]
    xk_t = [_tile_x(key[n]) for n in range(N_BATCH)]
    xv_t = [_tile_x(value[n]) for n in range(N_BATCH)]
    wo_t = np.ascontiguousarray(
        Wo.T.reshape(NK, 128, E).transpose(1, 0, 2)).astype(F16)
    ones = np.ones((128, 128), F16)
    in_maps = []
    cores_per_batch = N_CORES // N_BATCH
    for c in range(N_CORES):
        n = c // cores_per_batch
        hs = (c % cores_per_batch) * HDc
        sl = slice(hs, hs + HDc)
        in_maps.append({
            "xq": xq_t[n],
            "xk": xk_t[n],
            "xv": xv_t[n],
            "wq": _tile_w(Wq[sl]),
            "wk": _tile_w(Wk[sl]),
            "wv": _tile_w(Wv[sl]),
            "wo": wo_t,
            "bq": np.ascontiguousarray(bq[sl].reshape(NH, 128).T),
            "bk": np.ascontiguousarray(bk[sl].reshape(NH, 128).T),
            "bv": np.ascontiguousarray(bv[sl].reshape(NH, 128).T),
            "onf": ones,
        })
    return in_maps


def gather_outputs(cfg: Cfg, results):
    """results: list of per-core {'out': (NH*128, E)} -> full (N, S, E)."""
    E = cfg.E
    full = np.empty((N_BATCH, SEQ, E), np.float32)
    cores_per_batch = N_CORES // N_BATCH
    rows = cfg.NH * 128
    for c in range(N_CORES):
        n = c // cores_per_batch
        r0 = (c % cores_per_batch) * rows
        full[n, r0:r0 + rows, :] = results[c]["out"]
    return full


_CACHE = {}


def kernel(**inputs) -> np.ndarray:
    from concourse.bass_utils import run_bass_kernel_spmd
    cfg = Cfg()
    if "nc" not in _CACHE:
        _CACHE["nc"] = build_program(cfg)
    nc = _CACHE["nc"]
    in_maps = shard_inputs(cfg, **inputs)
    res = run_bass_kernel_spmd(nc, in_maps, core_ids=list(range(N_CORES)))
    full = gather_outputs(cfg, res.results)
    # bo is a pure affine epilogue: adding it here (fp32, exact) costs
    # nothing on-device
    full += np.asarray(inputs["bo"], np.float32).reshape(1, 1, cfg.E)
    return full


# revision 29
# speedup vs baseline: 1.0025x; 1.0025x over previous
# Multi-head attention (N=2, S=2048, E=2048, H=16, Dk=128) on 8 NeuronCores.
#
# Sharding: 2 batches x 16 heads = 32 (n,h) pairs -> core c owns batch c//4,
# heads (c%4)*4 .. +4. The reference reshapes (N,H,S,Dk)->(N,S,H*Dk) without
# a head transpose, so rows [h*128,(h+1)*128) of the pre-projection matrix X
# (and hence of the final output) depend on head h only: each core computes
# 512 disjoint output rows and the host concatenates. No collectives.
#
# v18 design - 404us on HW (v3 fp32r baseline: 542us). Where time goes:
# startup ~17us (DMA lead-in), projections ~170us (PE-bound), attention
# ~135us (Act/exp-bound, PE 93% busy under it), O-proj ~62us (PE-bound),
# drain ~8us. PE active ~368us vs 348us pure-stream floor.
#
#  - every matmul operand is fp16: same 1 cycle/row PE rate as fp32r and
#    bf16 (cost model: >=256 free dim), half the DMA/SBUF of fp32, 8x
#    finer mantissa than bf16. End-to-end mean rel err 2.1e-3 (bf16 gave
#    1.7e-2, right at the gate). PSUM accumulation is fp32 throughout.
#  - host pretiling: x inputs stream as [128p, k, 512s] chunk DMAs with
#    16KB contiguous per partition; weights are one [128, k, cols] DMA
#    each; ~50 DMA instructions total vs ~350 in v3 (each costs
#    0.6-0.8us of sequencer issue time).
#  - DMA completion-order control: queue packets stripe across all 16
#    DMA engines, so any concurrent transfer delays an earlier one's
#    completion semaphore, and the tile scheduler hoists every DMA whose
#    deps allow. Each x chunk is therefore pinned behind the previous
#    chunk's data via marker copies (one per DMA region, reading the
#    matching region of the predecessor), wk/wv are pinned behind xq0,
#    and the Wo prefetch is pinned behind the last v-group so it lands
#    in the attention phase where HBM is otherwise idle. Chunk 0 is
#    split across three queues for the fastest possible start.
#  - no DRAM spill: attention output oc and all of Wo (8MB fp16) stay in
#    SBUF; the O-projection runs with zero weight DMA.
#  - softmax denominator per (h,c) pair: two pairwise-add levels over
#    the 16 exp tiles (DVE 4+4 ops, GpSimd 4 ops - GpSimd cannot read
#    PSUM and costs ~1.3us per [128,512] op), then a 4-matmul PE chain
#    with a full ones[128,128] stationary, which yields the column sum
#    ALREADY BROADCAST to every partition (no [1,N] tile, no copy, no
#    separate broadcast matmul).
#  - bv is folded into the division epilogue: softmax rows sum to 1, so
#    attn@(v+bv) = attn@v + bv, and in the oc[d,h,s] layout bv is a
#    per-partition scalar -> one DVE tensor_scalar_add, zero PE cost.
#  - exp as [128,1024] ops over 2-bank PSUM score pairs; scores are
#    triple-buffered (6 banks) so PE is never gated by Act's drain rate;
#    outT/dbc accumulators are single-buffered (lifetimes don't overlap).
#    Act is the attention-phase bottleneck at ~8.3us/pair vs PE 7.7us.
#  - the first head-chunk's scores+exp interleave into the v-projection
#    to warm Act's pipeline (and its spline table loads at t=0 via a
#    warmup exp).
#  - O-proj loops h -> k -> nn: stationary (strided oc k-slice) loaded
#    once per (h,k), reused for 4 moving Wo tiles; bias via ones-row
#    matmul; outputs stream per 512-wide chunk.
#  - pe pstate is real: matmuls run ~634ns (1.2GHz) until ~3us of
#    continuous execution, so long stalls cost extra through re-ramp.
import numpy as np

F16 = np.float16

D_MODEL = 2048
NHEAD = 16
DK = 128
N_BATCH = 2
SEQ = 2048
N_CORES = 8
HEADS_PER_CORE = 4


class Cfg:
    def __init__(self, S=SEQ, E=D_MODEL, NH=HEADS_PER_CORE, CH=512):
        assert S % 128 == 0 and E % 128 == 0
        self.S = S          # sequence length
        self.E = E          # model dim (contraction for projections)
        self.NH = NH        # heads per core
        self.CH = CH        # s-chunk width for attention phase
        self.NK = E // 128  # contraction tiles for projections / O-proj
        self.NT = S // 128  # t tiles (attention contraction)
        self.HDc = NH * DK  # head dims per core
        self.NCH = S // CH  # number of attention s-chunks
        self.PCH = 512      # projection / O-proj free-dim chunk
        self.NPC = S // self.PCH   # projection s-chunks
        self.NOC = E // self.PCH   # O-proj output chunks
        assert S % CH == 0 and CH >= 256


def build_program(cfg: Cfg):
    import concourse.tile as tile
    from concourse import bacc, mybir
    from contextlib import ExitStack

    fp32 = mybir.dt.float32
    fp16 = mybir.dt.float16
    AF = mybir.ActivationFunctionType

    S, E, NH, CH = cfg.S, cfg.E, cfg.NH, cfg.CH
    NK, NT, HDc = cfg.NK, cfg.NT, cfg.HDc
    PCH, NPC, NOC, NCH = cfg.PCH, cfg.NPC, cfg.NOC, cfg.NCH
    inv_sqrt_dk = 1.0 / float(np.sqrt(DK))
    KH = NK // 2   # k-tiles per half DMA

    nc = bacc.Bacc("TRN2", target_bir_lowering=False, debug=False,
                   num_devices=N_CORES)

    # DRAM I/O (per-core values supplied via in_maps).
    # x inputs pretiled [s_chunk, partition, k, col]; weights [p, k, cols].
    xq = nc.dram_tensor("xq", [NPC, 128, NK, PCH], fp16,
                        kind="ExternalInput").ap()
    xk = nc.dram_tensor("xk", [NPC, 128, NK, PCH], fp16,
                        kind="ExternalInput").ap()
    xv = nc.dram_tensor("xv", [NPC, 128, NK, PCH], fp16,
                        kind="ExternalInput").ap()
    wq = nc.dram_tensor("wq", [128, NK, HDc], fp16, kind="ExternalInput").ap()
    wk = nc.dram_tensor("wk", [128, NK, HDc], fp16, kind="ExternalInput").ap()
    wv = nc.dram_tensor("wv", [128, NK, HDc], fp16, kind="ExternalInput").ap()
    wo = nc.dram_tensor("wo", [128, NK, E], fp16, kind="ExternalInput").ap()
    bq = nc.dram_tensor("bq", [128, NH], fp32, kind="ExternalInput").ap()
    bk = nc.dram_tensor("bk", [128, NH], fp32, kind="ExternalInput").ap()
    bv = nc.dram_tensor("bv", [128, NH], fp32, kind="ExternalInput").ap()
    onf = nc.dram_tensor("onf", [128, 128], fp16, kind="ExternalInput").ap()
    out = nc.dram_tensor("out", [NH * 128, E], fp32,
                         kind="ExternalOutput").ap()

    with tile.TileContext(nc) as tc, ExitStack() as top:
        persist = top.enter_context(tc.tile_pool(name="persist", bufs=1))
        qc = persist.tile([128, NH, S], fp16, name="qc")   # qT: [d, h, s]
        kc = persist.tile([128, NH, S], fp16, name="kc")   # kT: [d, h, s]
        vc = persist.tile([128, NT, HDc], fp16, name="vc")  # [t_p, t_t, hd]
        # Wo cache in two halves: the second half's SBUF is only
        # allocated from phase B on (after the xin pool closes)
        wop = top.enter_context(tc.tile_pool(name="wop", bufs=1))
        wo_a = wop.tile([128, NK // 2, E], fp16, name="wo_a")

        consts = top.enter_context(tc.tile_pool(name="consts", bufs=1))
        ones_sb = consts.tile([128, 128], fp16, name="ones")
        bq_sb = consts.tile([128, NH], fp32, name="bq")
        bk_sb = consts.tile([128, NH], fp32, name="bk")
        bv_sb = consts.tile([128, NH], fp32, name="bv")
        warm = consts.tile([128, 1], fp32, name="warm")

        # SBUF/PSUM pools release LIFO per side; phase-spanning pools go on
        # the right side so phase-local left pools can close under them.
        es_a = ExitStack()   # xin (left)
        es_qk = ExitStack()  # wq/wk sbuf + qk psum (left)
        es_b = ExitStack()   # expp + scores psum (right; A2 through B)
        es_v = ExitStack()   # wv sbuf + v psum (left)
        es_oc = ExitStack()  # oc + attn smalls (left; B + C)
        es_ps = ExitStack()  # ot/dn psum (left; B only)
        es_c = ExitStack()   # osb + o-proj psum (right)

        # ============== Phase A1: q/k projections ==============
        xin = es_a.enter_context(tc.tile_pool(name="xin", bufs=3))
        wqk = es_qk.enter_context(tc.tile_pool(name="wqk", bufs=1))
        pa = es_qk.enter_context(tc.tile_pool(name="pa", bufs=2,
                                              space="PSUM"))
        wq_sb = wqk.tile([128, NK, HDc], fp16, name="wq_sb")
        wk_sb = wqk.tile([128, NK, HDc], fp16, name="wk_sb")
        # only wq + xq0 + consts move at t=0: everything else is pinned
        # behind xq0's arrival (marker copies) so the first s-chunk's data
        # isn't delayed by unrelated packets striped onto the same engines
        for hf in range(2):
            ks = slice(hf * KH, (hf + 1) * KH)
            nc.gpsimd.dma_start(wq_sb[:, ks, :], wq[:, ks, :])
        for t, d in ((bq_sb, bq), (bk_sb, bk), (bv_sb, bv),
                     (ones_sb, onf)):
            nc.gpsimd.dma_start(t[:], d)
        # trigger the Exp table load while the weight/x DMAs stream
        nc.scalar.activation(warm[:], bq_sb[:, :1], AF.Exp)

        xeng = [nc.sync, nc.scalar, nc.gpsimd]
        XSPLIT = [0, 6, 11, NK]   # thirds across three queues
        pin_hist = []

        def load_x(x_d, s):
            # chunk 0 is split over three queues (fastest possible start);
            # later chunks are ONE full DMA each (16KB contiguous per
            # partition -> 16KB packets, ~2x the per-packet bandwidth of
            # half-chunk 8KB packets), alternating queues, each pinned
            # behind the previous chunk's data - queue packets stripe over
            # shared engines, so unordered chunks delay each other's
            # completion semaphores.
            t = xin.tile([128, NK, PCH], fp16, tag="xin")
            if not pin_hist:
                for i in range(3):
                    ks = slice(XSPLIT[i], XSPLIT[i + 1])
                    xeng[i].dma_start(t[:, ks, :], x_d[s][:, ks, :])
            else:
                nc.vector.tensor_copy(t[:1, 0, :2], pin_hist[-1][:1, 0, :2])
                xeng[len(pin_hist) % 2].dma_start(t[:], x_d[s])
            pin_hist.append(t)
            return t

        def proj_qk(x_d, w_sb, bias_sb, dst):
            for s in range(NPC):
                first = not pin_hist
                xt = load_x(x_d, s)
                if first:
                    for i in range(3):
                        nc.vector.tensor_copy(
                            wk_sb[:1, XSPLIT[i], :2], xt[:1, XSPLIT[i], :2])
                    for hf in range(2):
                        ks = slice(hf * KH, (hf + 1) * KH)
                        nc.gpsimd.dma_start(wk_sb[:, ks, :], wk[:, ks, :])

                ps = [pa.tile([128, PCH], fp32, tag=f"pa{m}", name=f"pa{m}")
                      for m in range(NH)]
                for m in range(NH):
                    for k in range(NK):
                        nc.tensor.matmul(
                            ps[m][:], w_sb[:, k, m * 128:(m + 1) * 128],
                            xt[:, k, :], start=(k == 0), stop=(k == NK - 1))
                for m in range(NH):
                    # GpSimd cannot read PSUM; evict on DVE + Act
                    if m % 2 == 0:
                        nc.vector.tensor_scalar_add(
                            dst[:, m, s * PCH:(s + 1) * PCH], ps[m][:],
                            bias_sb[:, m:m + 1])
                    else:
                        nc.scalar.activation(
                            dst[:, m, s * PCH:(s + 1) * PCH], ps[m][:],
                            AF.Identity, bias=bias_sb[:, m:m + 1])

        proj_qk(xq, wq_sb, bq_sb, qc)
        proj_qk(xk, wk_sb, bk_sb, kc)
        es_qk.close()

        # pools that span A2 + all of phase B
        expp = es_b.enter_context(tc.tile_pool(name="expp", bufs=2,
                                               side="right"))
        stp = [None]  # scores psum pool: tmp 4-bank pool in A2, 6-bank in B

        def scores_exp(h, c):
            cs = slice(c * CH, (c + 1) * CH)
            expT = expp.tile([128, NT, CH], fp16, tag="expT",
                             name=f"expT_{h}_{c}")
            for tp in range(NT // 2):
                ps = stp[0].tile([128, 2 * CH], fp32, tag="st", name="st")
                for half in range(2):
                    tt = tp * 2 + half
                    nc.tensor.matmul(
                        ps[:, half * CH:(half + 1) * CH],
                        kc[:, h, tt * 128:(tt + 1) * 128],
                        qc[:, h, cs], start=True, stop=True)
                nc.scalar.activation(
                    expT[:, tp * 2:tp * 2 + 2, :].rearrange(
                        "p a b -> p (a b)"),
                    ps[:], AF.Exp, scale=inv_sqrt_dk)
            return expT

        # ===== Phase A2: v projection, first scores interleaved =====
        wvp = es_v.enter_context(tc.tile_pool(name="wvp", bufs=1))
        vps = es_v.enter_context(tc.tile_pool(name="vps", bufs=1,
                                              space="PSUM"))
        stp[0] = es_v.enter_context(tc.tile_pool(name="st0", bufs=2,
                                                 space="PSUM"))
        wv_sb = wvp.tile([128, NK, HDc], fp16, name="wv_sb")
        for hf in range(2):
            nc.vector.tensor_copy(wv_sb[:1, hf * KH, :2],
                                  pin_hist[-1][:1, 0, :2])
            ks = slice(hf * KH, (hf + 1) * KH)
            nc.gpsimd.dma_start(wv_sb[:, ks, :], wv[:, ks, :])

        def proj_v_group(g):
            xt = load_x(xv, g)
            ps = [vps.tile([128, HDc], fp32, tag=f"pv{j}", name=f"pv{j}")
                  for j in range(4)]
            for k in range(NK):
                for j in range(4):
                    nc.tensor.matmul(
                        ps[j][:], xt[:, k, j * 128:(j + 1) * 128],
                        wv_sb[:, k, :], start=(k == 0), stop=(k == NK - 1))
            for j in range(4):
                if j % 2 == 0:
                    nc.vector.tensor_copy(vc[:, g * 4 + j, :], ps[j][:])
                else:
                    nc.scalar.activation(vc[:, g * 4 + j, :], ps[j][:],
                                         AF.Identity)

        pend = [(0, 0, scores_exp(0, 0))]   # PE fills the xv0 DMA wait
        proj_v_group(0)
        proj_v_group(1)
        proj_v_group(2)
        proj_v_group(3)
        es_v.close()
        es_a.close()
        stp[0] = es_b.enter_context(tc.tile_pool(name="st2", bufs=3,
                                                 space="PSUM", side="right"))

        # ============== Phase B: attention ==============
        ocp = es_oc.enter_context(tc.tile_pool(name="ocp", bufs=1))
        oc = ocp.tile([128, NH, S], fp16, name="oc")  # attn out [d, h, s]
        wo_b = ocp.tile([128, NK // 2, E], fp16, name="wo_b")
        bsc = es_oc.enter_context(tc.tile_pool(name="bsc", bufs=2))
        otp = es_ps.enter_context(tc.tile_pool(name="otp", bufs=1,
                                               space="PSUM"))
        dnp = es_ps.enter_context(tc.tile_pool(name="dnp", bufs=1,
                                               space="PSUM"))

        # prefetch all of Wo (8MB fp16) during attention. The marker copy
        # (depends on vc's last v-group) pins the DMAs to the start of
        # phase B - without it the scheduler hoists them into phase A's
        # DMA window where they fight the x loads for HBM bandwidth.
        woq = [nc.sync, nc.gpsimd, nc.sync, nc.gpsimd]
        for i in range(4):
            dst = wo_a if i < 2 else wo_b
            j = i % 2
            nc.vector.tensor_copy(dst[:1, j * 4, :2], vc[:1, NT - 1, :2])
            woq[i].dma_start(dst[:, j * 4:(j + 1) * 4, :],
                             wo[:, i * 4:(i + 1) * 4, :])

        def attn_tail(h, c, expT, first=False):
            cs = slice(c * CH, (c + 1) * CH)
            op = otp.tile([128, CH], fp32, tag="ot", name="ot")
            for tt in range(NT):
                nc.tensor.matmul(
                    op[:], vc[:, tt, h * 128:(h + 1) * 128],
                    expT[:, tt, :], start=(tt == 0), stop=(tt == NT - 1))
            dbc = dnp.tile([128, CH], fp32, tag="dbc", name="dbc")
            if first:
                # PE idles at B start waiting on the DVE/GP tree; a direct
                # 16-matmul ones chain fills that window instead
                for tt in range(NT):
                    nc.tensor.matmul(dbc[:], ones_sb[:], expT[:, tt, :],
                                     start=(tt == 0), stop=(tt == NT - 1))
            else:
                # denominator: 2-level pairwise reduce (8 then 4 tiles),
                # then a 4-matmul full-ones chain = column sum broadcast
                # to all partitions in PSUM
                acc = bsc.tile([128, 8, CH], fp16, tag="acc", name="acc")
                for i in range(4):
                    nc.vector.tensor_add(acc[:, i, :], expT[:, 2 * i, :],
                                         expT[:, 2 * i + 1, :])
                for i in range(4, 8):
                    nc.gpsimd.tensor_add(acc[:, i, :], expT[:, 2 * i, :],
                                         expT[:, 2 * i + 1, :])
                for j in range(4):
                    nc.vector.tensor_add(acc[:, j, :], acc[:, 2 * j, :],
                                         acc[:, 2 * j + 1, :])
                for j in range(4):
                    nc.tensor.matmul(dbc[:], ones_sb[:], acc[:, j, :],
                                     start=(j == 0), stop=(j == 3))
            rsc = bsc.tile([128, CH], fp32, tag="rsc", name="rsc")
            nc.vector.reciprocal_approx_fast(rsc[:], dbc[:])
            nc.vector.tensor_mul(oc[:, h, cs], op[:], rsc[:])
            nc.vector.tensor_scalar_add(oc[:, h, cs], oc[:, h, cs],
                                        bv_sb[:, h:h + 1])

        pairs = [(h, c) for h in range(NH) for c in range(NCH)]
        first_tail = [True]
        for h, c in pairs[1:]:
            expT = scores_exp(h, c)
            attn_tail(*pend.pop(), first=first_tail[0])
            first_tail[0] = False
            pend.append((h, c, expT))
        attn_tail(*pend.pop())
        es_ps.close()
        es_b.close()

        # ============== Phase C: output projection ==============
        osb = es_c.enter_context(tc.tile_pool(name="osb", bufs=2,
                                              side="right"))
        pcp = es_c.enter_context(tc.tile_pool(name="pcp", bufs=2,
                                              space="PSUM", side="right"))
        for h in range(NH):
            ops = [pcp.tile([128, PCH], fp32, tag=f"pc{nn}", name=f"pc{nn}")
                   for nn in range(NOC)]
            # lhsT = X_h^T k-tile: strided view of oc (s = j*NK + k)
            lhs_h = oc[:, h, :].rearrange("p (j i) -> p i j", i=NK)
            for k in range(NK):
                wt = wo_a if k < NK // 2 else wo_b
                kk = k % (NK // 2)
                for nn in range(NOC):
                    nc.tensor.matmul(
                        ops[nn][:], lhs_h[:, k, :],
                        wt[:, kk, nn * PCH:(nn + 1) * PCH],
                        start=(k == 0), stop=(k == NK - 1))
            ot = osb.tile([128, E], fp32, tag="osb", name="osb")
            for nn in range(NOC):
                ns = slice(nn * PCH, (nn + 1) * PCH)
                if nn % 2 == 0:
                    nc.vector.tensor_copy(ot[:, ns], ops[nn][:])
                else:
                    nc.scalar.activation(ot[:, ns], ops[nn][:], AF.Identity)
                nc.sync.dma_start(out[h * 128:(h + 1) * 128, ns],
                                  ot[:, ns])
        es_c.close()
        es_oc.close()

    nc.compile()
    return nc


def _tile_x(x):
    # (S, E) -> [s_chunk, partition, k, col] fp16, 16KB contiguous per
    # partition per chunk
    S, E = x.shape
    return np.ascontiguousarray(
        x.reshape(S // 512, 512, E // 128, 128).transpose(0, 3, 2, 1)
    ).astype(F16)


def _tile_w(w_slice):
    # (HDc, E) -> [partition, k, col] fp16
    HDc, E = w_slice.shape
    return np.ascontiguousarray(
        w_slice.T.reshape(E // 128, 128, HDc).transpose(1, 0, 2)
    ).astype(F16)


def shard_inputs(cfg: Cfg, query, key, value, Wq, bq, Wk, bk, Wv, bv, Wo, bo):
    """Build per-core in_maps from full inputs."""
    f = np.float32
    query, key, value = (np.asarray(a, f) for a in (query, key, value))
    Wq, Wk, Wv, Wo = (np.asarray(a, f) for a in (Wq, Wk, Wv, Wo))
    bq, bk, bv, bo = (np.asarray(a, f) for a in (bq, bk, bv, bo))
    NH, HDc, NK, E = cfg.NH, cfg.HDc, cfg.NK, cfg.E
    xq_t = [_tile_x(query[n]) for n in range(N_BATCH)]
    xk_t = [_tile_x(key[n]) for n in range(N_BATCH)]
    xv_t = [_tile_x(value[n]) for n in range(N_BATCH)]
    wo_t = np.ascontiguousarray(
        Wo.T.reshape(NK, 128, E).transpose(1, 0, 2)).astype(F16)
    ones = np.ones((128, 128), F16)
    in_maps = []
    cores_per_batch = N_CORES // N_BATCH
    for c in range(N_CORES):
        n = c // cores_per_batch
        hs = (c % cores_per_batch) * HDc
        sl = slice(hs, hs + HDc)
        in_maps.append({
            "xq": xq_t[n],
            "xk": xk_t[n],
            "xv": xv_t[n],
            "wq": _tile_w(Wq[sl]),
            "wk": _tile_w(Wk[sl]),
            "wv": _tile_w(Wv[sl]),
            "wo": wo_t,
            "bq": np.ascontiguousarray(bq[sl].reshape(NH, 128).T),
            "bk": np.ascontiguousarray(bk[sl].reshape(NH, 128).T),
            "bv": np.ascontiguousarray(bv[sl].reshape(NH, 128).T),
            "onf": ones,
        })
    return in_maps


def gather_outputs(cfg: Cfg, results):
    """results: list of per-core {'out': (NH*128, E)} -> full (N, S, E)."""
    E = cfg.E
    full = np.empty((N_BATCH, SEQ, E), np.float32)
    cores_per_batch = N_CORES // N_BATCH
    rows = cfg.NH * 128
    for c in range(N_CORES):
        n = c // cores_per_batch
        r0 = (c % cores_per_batch) * rows
        full[n, r0:r0 + rows, :] = results[c]["out"]
    return full


_CACHE = {}


def kernel(**inputs) -> np.ndarray:
    from concourse.bass_utils import run_bass_kernel_spmd
    cfg = Cfg()
    if "nc" not in _CACHE:
        _CACHE["nc"] = build_program(cfg)
    nc = _CACHE["nc"]
    in_maps = shard_inputs(cfg, **inputs)
    res = run_bass_kernel_spmd(nc, in_maps, core_ids=list(range(N_CORES)))
    full = gather_outputs(cfg, res.results)
    # bo is a pure affine epilogue: adding it here (fp32, exact) costs
    # nothing on-device
    full += np.asarray(inputs["bo"], np.float32).reshape(1, 1, cfg.E)
    return full
